# revision 4
# baseline (speedup 1.0000x reference)
"""Multi-head attention (B=2, S=4096, D=768, H=12) on 8 Trainium2 cores.

Sharding: (batch, head-group) -> core.  Core c handles batch c//4 and heads
3*(c%4) .. 3*(c%4)+2.  Q/K/V projections are computed per-core on the head
slice of the weights; the output projection is computed as a partial product
over the core's 192 combined-head dims and the 4 partials per batch are summed
on the host (the "all-reduce").

Device-side design (v2):
  - PE stream-rows are the floor (~410us of matmul streaming); ACT exp was
    the secondary floor (427us).  v2 attacks both:
      * ~25% of the softmax exp tiles move to the Vector engine via a
        one-instruction Schraudolph exp2 bit-trick: scores carry
        y = log2e * s (log2e/8 folded into Wq), probs_bf16 = bitcast_i16(
        y*128 + 16248.75).  ACT tiles compute exp(y*ln2) via scale=ln2.
        rms rel err of the trick is 1.8%; at 25% of tiles it adds ~0.9%
        in quadrature to the output.
      * the epilogue is transpose-free: attn^T [dk,q] tiles are normalized
        in place by broadcasting 1/den across partitions with tiny K=1
        PE matmuls (den rows staged to SBUF by DMA, one native reciprocal
        per 512-chunk), and the output projection runs transposed
        (out^T[od,s] = WoT-chunk @ attn^T) writing partT [768, S].
        This kills all 160 PE transposes and most epilogue DVE traffic.
      * q/k projections stream N=512 (vs 256) so LDWEIGHTS hides.
  - Everything else is inherited from v1: flat (qc, h, pair) loop with
    cross-boundary scores prefetch, filler pump for projections/epilogue,
    3 parallel DMA queues for inputs, row-packed scores via tile_position,
    v carries a ones-column so attn^T row 64 accumulates the denominator.
  - PSUM: 2x scores(2 banks) + 2x acc(1) + 2x misc(1) = 8 banks.
"""

import os
import sys
from collections import deque

import numpy as np

for _p in ("/opt/trn_rl_repo", "/root/.axon_site/_ro/trn_rl_repo"):
    if _p not in sys.path and os.path.isdir(_p):
        sys.path.append(_p)

import concourse.bass as bass
import concourse.mybir as mybir
import concourse.tile as tile
from concourse.bass_utils import run_bass_kernel_spmd

try:
    from ml_dtypes import bfloat16 as _bf16np
except ImportError:  # pragma: no cover
    _bf16np = np.dtype("bfloat16").type

F32 = mybir.dt.float32
BF16 = mybir.dt.bfloat16
I16 = mybir.dt.int16

D_MODEL = 768
N_HEADS_CORE = 3  # heads per core
DH = 192  # N_HEADS_CORE * 64
KCH = D_MODEL // 128  # contraction chunks for projections

LOG2E = 1.4426950408889634
LN2 = 0.6931471805599453
B_EXP2 = 127.0 * 128.0 - 7.25  # Schraudolph constant for bf16 exp2 trick
PART_BF16 = True  # bf16 partials halve the output-DMA tail


def split_multi_waits(nc, max_waits=1):
    """This container's walrus rejects >1 semaphore wait per instruction
    (setupSyncWait).  Move excess waits onto same-engine NoOps just before
    the offending instruction."""
    n = 0
    for f in nc.m.functions:
        for bb in f.blocks:
            out = []
            for inst in bb.instructions:
                si = inst.sync_info
                if si is not None and si.on_wait and len(si.on_wait) > max_waits:
                    waits = list(si.on_wait)
                    for j, w in enumerate(waits[:-max_waits]):
                        out.append(
                            mybir.InstNoOp(
                                name=f"{inst.name}-wsplit{j}",
                                engine=inst.engine,
                                ins=[],
                                outs=[],
                                sync_info=mybir.SyncInfo(on_wait=[w], on_update=[]),
                            )
                        )
                    si.on_wait = waits[-max_waits:]
                    n += 1
                out.append(inst)
            bb.instructions = out
    return n


def build_nc(S, has_bq=True, has_bk=True, split=True):
    assert S % 512 == 0
    NQ = S // 512  # query chunks / projection chunks
    NT = S // 128  # kpos tiles
    NP2 = NT // 2  # kpos tile pairs for the h2 phase
    nc = bass.Bass()
    PDT = BF16 if PART_BF16 else F32

    # chunk-contiguous layouts: one 6KB descriptor per partition per chunk
    xqa = nc.declare_dram_parameter("xqa", [S // 512, 128, KCH, 512], BF16, isOutput=False)
    xka = nc.declare_dram_parameter("xka", [S // 512, 128, KCH, 512], BF16, isOutput=False)
    xva = nc.declare_dram_parameter("xva", [S // 512, 128, KCH, 512], BF16, isOutput=False)
    wqa = nc.declare_dram_parameter("wqa", [128, KCH, DH], BF16, isOutput=False)
    wka = nc.declare_dram_parameter("wka", [128, KCH, DH], BF16, isOutput=False)
    wva = nc.declare_dram_parameter("wva", [128, KCH, DH], BF16, isOutput=False)
    wo0 = nc.declare_dram_parameter("wo0", [128, D_MODEL], BF16, isOutput=False)
    wo1 = nc.declare_dram_parameter("wo1", [64, D_MODEL], BF16, isOutput=False)
    bq = nc.declare_dram_parameter("bq", [DH, 1], F32, isOutput=False) if has_bq else None
    bk = nc.declare_dram_parameter("bk", [DH, 1], F32, isOutput=False) if has_bk else None
    part = nc.declare_dram_parameter("part", [D_MODEL, S], PDT, isOutput=True)

    with tile.TileContext(nc) as tc:
        with (
            tc.tile_pool(name="consts", bufs=1) as consts,
            tc.tile_pool(name="persist", bufs=1) as persist,
            tc.tile_pool(name="xin", bufs=1) as xin,
            tc.tile_pool(name="probs", bufs=6) as probs_pool,
            tc.tile_pool(name="epi_sb", bufs=2) as epi_sb,
        ):
            # ---- constants ----
            wq_sb = consts.tile([128, KCH, DH], BF16, tag="wq_sb")
            nc.scalar.dma_start(out=wq_sb, in_=wqa[:, :, :])
            wk_sb = consts.tile([128, KCH, DH], BF16, tag="wk_sb")
            nc.scalar.dma_start(out=wk_sb, in_=wka[:, :, :])
            wv_sb = consts.tile([128, KCH, DH], BF16, tag="wv_sb")
            nc.scalar.dma_start(out=wv_sb, in_=wva[:, :, :])
            bq_lo = bq_hi = bk_lo = bk_hi = None
            if has_bq:
                bq_lo = consts.tile([128, 1], F32, tag="bq_lo")
                nc.scalar.dma_start(out=bq_lo, in_=bq[0:128, :])
                bq_hi = consts.tile([64, 1], F32, tag="bq_hi")
                nc.scalar.dma_start(out=bq_hi, in_=bq[128:DH, :])
            if has_bk:
                bk_lo = consts.tile([128, 1], F32, tag="bk_lo")
                nc.scalar.dma_start(out=bk_lo, in_=bk[0:128, :])
                bk_hi = consts.tile([64, 1], F32, tag="bk_hi")
                nc.scalar.dma_start(out=bk_hi, in_=bk[128:DH, :])
            wo0_sb = consts.tile([128, D_MODEL], BF16, tag="wo0")
            wo1_sb = consts.tile([64, D_MODEL], BF16, tag="wo1")
            # ones rows (at 32-aligned partitions) for the 1/den broadcast mms
            sel = consts.tile([65, 64], BF16, tag="sel")
            nc.vector.memset(sel[0:1, :], 1.0)
            nc.vector.memset(sel[32:33, :], 1.0)
            nc.vector.memset(sel[64:65, :], 1.0)

            # ---- persistent activations, per 512-col chunk ----
            qTr = [[persist.tile([128, 512], BF16, tag=f"qTr{h}_{c}", name=f"qTr{h}_{c}")
                    for c in range(NQ)] for h in range(3)]
            kTr = [[persist.tile([128, 512], BF16, tag=f"kTr{h}_{c}", name=f"kTr{h}_{c}")
                    for c in range(NQ)] for h in range(3)]
            v_c = [persist.tile([128, 4, 3, 65], BF16, tag=f"v_{c}", name=f"v_{c}")
                   for c in range(NQ)]
            for c in range(NQ):
                nc.vector.memset(v_c[c][:, :, :, 64:65], 1.0)

            # per-chunk x views: (tile, column offset within tile)
            xq_t = [None] * NQ
            xk_t = [None] * NQ
            xv_t = [None] * NQ

            def dma_xq(c):
                t = xin.tile([128, KCH, 512], BF16, tag="xq", bufs=2, name=f"xq{c}")
                nc.sync.dma_start(out=t, in_=xqa[c])
                xq_t[c] = (t, 0)

            def dma_xk(c, eng=None):
                t = xin.tile([128, KCH, 512], BF16, tag="xk1", bufs=4, name=f"xk{c}")
                (eng or nc.gpsimd).dma_start(out=t, in_=xka[c])
                xk_t[c] = (t, 0)

            def dma_xv(c, eng=None):
                t = xin.tile([128, KCH, 512], BF16, tag="xv1", bufs=4, name=f"xv{c}")
                (eng or nc.sync).dma_start(out=t, in_=xva[c])
                xv_t[c] = (t, 0)

            with (
                tc.tile_pool(name="ps_big", bufs=2, space="PSUM") as ps_big,
                tc.tile_pool(name="ps_acc", bufs=2, space="PSUM") as ps_acc,
                tc.tile_pool(name="ps_misc", bufs=2, space="PSUM") as ps_misc,
            ):
                # ---------- projection helpers ----------
                def qk_proj_units(w_sb, xv, dst, blo, bhi, c):
                    """One full 512-col q/k projection chunk; N=512 streams so
                    LDWEIGHTS hides behind the previous matmul."""
                    x_t, xoff = xv
                    xc = bass.ds(xoff, 512)
                    ps0 = ps_misc.tile([128, 512], F32, tag="misc", name="ps0")
                    for k0 in range(0, KCH, 2):
                        for kk in (k0, k0 + 1):
                            nc.tensor.matmul(ps0, w_sb[:, kk, 0:128], x_t[:, kk, xc],
                                             start=(kk == 0), stop=(kk == KCH - 1),
                                             skip_group_check=True)
                        yield
                    ps1 = ps_misc.tile([128, 512], F32, tag="misc", name="ps1")
                    for k0 in range(0, KCH, 2):
                        for kk in (k0, k0 + 1):
                            nc.tensor.matmul(ps1[0:64, :], w_sb[:, kk, 128:DH], x_t[:, kk, xc],
                                             start=(kk == 0), stop=(kk == KCH - 1),
                                             skip_group_check=True)
                        yield
                    if blo is not None:
                        nc.vector.tensor_scalar_add(dst[0][c][0:64, :], ps0[0:64, :], blo[0:64])
                        nc.vector.tensor_scalar_add(dst[1][c][64:128, :], ps0[64:128, :], blo[64:128])
                        nc.vector.tensor_scalar_add(dst[2][c][0:64, :], ps1[0:64, :], bhi[0:64])
                    else:
                        nc.vector.tensor_copy(dst[0][c][0:64, :], ps0[0:64, :])
                        nc.vector.tensor_copy(dst[1][c][64:128, :], ps0[64:128, :])
                        nc.vector.tensor_copy(dst[2][c][0:64, :], ps1[0:64, :])
                    nc.sync.dma_start(out=dst[2][c][64:128, :], in_=dst[2][c][0:64, :])
                    yield

                kproj_done = [False] * NQ
                vproj_done = [False] * NQ
                qproj_done = [False] * NQ

                def kproj_units(c):
                    yield from qk_proj_units(wk_sb, xk_t[c], kTr, bk_lo, bk_hi, c)
                    kproj_done[c] = True

                def qproj_units(c):
                    yield from qk_proj_units(wq_sb, xq_t[c], qTr, bq_lo, bq_hi, c)
                    qproj_done[c] = True

                def vproj_units(c):
                    x_t, xoff = xv_t[c]
                    for sub in range(4):
                        vps = ps_misc.tile([128, 512], F32, tag="misc", name="vps")
                        for kk in range(KCH):
                            nc.tensor.matmul(
                                vps[:, 0:DH],
                                x_t[:, kk, bass.ds(xoff + sub * 128, 128)],
                                wv_sb[:, kk, :],
                                start=(kk == 0), stop=(kk == KCH - 1),
                                skip_group_check=True,
                            )
                        nc.vector.tensor_copy(
                            v_c[c][:, sub, :, 0:64],
                            vps[:, 0:DH].rearrange("p (h d) -> p h d", h=3),
                        )
                        yield
                    vproj_done[c] = True

                # ---------- epilogue (transpose-free) ----------
                catn_t = {}

                def epi_norm_units(qc, den_st, cat0, cat1):
                    rec = epi_sb.tile([65, 512], F32, tag="rec", name="rec")
                    nc.vector.reciprocal(rec, den_st)
                    yield
                    recb = epi_sb.tile([65, 512], BF16, tag="recb", name="recb")
                    nc.vector.tensor_copy(recb, rec)
                    yield
                    bc0 = ps_misc.tile([128, 512], F32, tag="misc", name="bc0")
                    nc.tensor.matmul(bc0[0:64, :], sel[0:1, :], recb[0:1, :],
                                     start=True, stop=True, skip_group_check=True)
                    nc.tensor.matmul(bc0[64:128, :], sel[32:33, :], recb[32:33, :],
                                     start=True, stop=True, skip_group_check=True)
                    yield
                    catn0 = epi_sb.tile([128, 512], BF16, tag="catn0", name="catn0")
                    nc.vector.tensor_tensor(out=catn0, in0=cat0, in1=bc0,
                                            op=mybir.AluOpType.mult)
                    yield
                    bc1 = ps_misc.tile([128, 512], F32, tag="misc", name="bc1")
                    nc.tensor.matmul(bc1[0:64, :], sel[64:65, :], recb[64:65, :],
                                     start=True, stop=True, skip_group_check=True)
                    catn1 = epi_sb.tile([64, 512], BF16, tag="catn1", name="catn1")
                    nc.vector.tensor_tensor(out=catn1, in0=cat1, in1=bc1[0:64, :],
                                            op=mybir.AluOpType.mult)
                    catn_t[qc] = (catn0, catn1)
                    yield

                out_queues = None  # set below

                def epi_out_units(qc):
                    catn0, catn1 = catn_t.pop(qc)
                    for t in range(6):
                        opT = ps_misc.tile([128, 512], F32, tag="misc", name="opT")
                        nc.tensor.matmul(opT, wo0_sb[:, bass.ts(t, 128)], catn0,
                                         start=True, stop=False, skip_group_check=True)
                        nc.tensor.matmul(opT, wo1_sb[:, bass.ts(t, 128)], catn1,
                                         start=False, stop=True, skip_group_check=True)
                        o_sb = epi_sb.tile([128, 512], PDT, tag="osb", bufs=4, name="osb")
                        nc.vector.tensor_copy(o_sb, opT)
                        q = out_queues[(qc * 6 + t) % len(out_queues)]
                        q.dma_start(
                            out=part[t * 128:(t + 1) * 128, qc * 512:(qc + 1) * 512],
                            in_=o_sb,
                        )
                        yield

                out_queues = [nc.sync, nc.gpsimd, nc.scalar]

                # ---------- scores ----------
                def scores_h01(t, qc):
                    sc = ps_big.tile([128, 1024], F32, tag="big", name="sc")
                    tc_ = bass.ts(t % 4, 128)
                    nc.tensor.matmul(
                        sc[:, 0:512], kTr[0][t // 4][0:64, tc_], qTr[0][qc][0:64, :],
                        start=True, stop=True, tile_position=(0, 0),
                    )
                    nc.tensor.matmul(
                        sc[:, 512:1024], kTr[1][t // 4][64:128, tc_], qTr[1][qc][64:128, :],
                        start=True, stop=True, tile_position=(64, 0),
                    )
                    return sc

                def scores_h2(t2, qc):
                    t0, t1 = 2 * t2, 2 * t2 + 1
                    sc = ps_big.tile([128, 1024], F32, tag="big", name="sc")
                    nc.tensor.matmul(
                        sc[:, 0:512],
                        kTr[2][t0 // 4][0:64, bass.ts(t0 % 4, 128)],
                        qTr[2][qc][0:64, :],
                        start=True, stop=True, tile_position=(0, 0),
                    )
                    nc.tensor.matmul(
                        sc[:, 512:1024],
                        kTr[2][t1 // 4][64:128, bass.ts(t1 % 4, 128)],
                        qTr[2][qc][64:128, :],
                        start=True, stop=True, tile_position=(64, 0),
                    )
                    return sc

                # ---------- filler pump ----------
                fq = deque()

                def pump(n):
                    done = 0
                    while done < n and fq:
                        try:
                            next(fq[0])
                            done += 1
                        except StopIteration:
                            fq.popleft()

                def drain_until(flags, c):
                    while not flags[c] and fq:
                        try:
                            next(fq[0])
                        except StopIteration:
                            fq.popleft()
                    assert flags[c], f"filler queue drained but chunk {c} not emitted"

                # ---------- startup ----------
                dma_xq(0)
                queues = [nc.gpsimd, nc.sync, nc.scalar]
                qi = 0
                for c in range(NQ):
                    dma_xk(c, queues[qi % 3]); qi += 1
                    dma_xv(c, queues[qi % 3]); qi += 1
                for g in (qproj_units(0), kproj_units(0)):
                    for _ in g:
                        pass
                fq.append(vproj_units(0))
                fq.append(kproj_units(1))
                fq.append(vproj_units(1))
                for c in range(2, NQ):
                    fq.append(kproj_units(c))
                    fq.append(vproj_units(c))

                # ---------- flat main loop ----------
                seq = []
                for qc in range(NQ):
                    seq.append((qc, "h01"))
                    seq.append((qc, "h2"))

                sc_cur = None
                den_cur = [None]
                cat0_cur = [None]

                def emit_scores(qc, phase, i):
                    if phase == "h01":
                        drain_until(kproj_done, i // 4)
                        drain_until(qproj_done, qc)
                        return scores_h01(i, qc)
                    drain_until(kproj_done, (2 * i) // 4)
                    drain_until(kproj_done, (2 * i + 1) // 4)
                    drain_until(qproj_done, qc)
                    return scores_h2(i, qc)

                sc_cur = emit_scores(0, "h01", 0)
                for si, (qc, phase) in enumerate(seq):
                    npair = NT if phase == "h01" else NP2
                    if phase == "h01":
                        acc0 = ps_acc.tile([65, 512], F32, tag="acc", name="acc0")
                        acc1 = ps_acc.tile([65, 512], F32, tag="acc", name="acc1")
                        pump_n = 6 if qc == 0 else 1
                        if qc >= 1 and qc + 1 < NQ:
                            dma_xq(qc + 1)
                            fq.append(qproj_units(qc + 1))
                    else:
                        acc2 = ps_acc.tile([65, 512], F32, tag="acc", name="acc2")
                        pump_n = 3 if qc == 0 else 1
                        if qc == 0:
                            dma_xq(1)
                            fq.append(qproj_units(1))
                            nc.sync.dma_start(out=wo0_sb, in_=wo0[:, :])
                            nc.sync.dma_start(out=wo1_sb, in_=wo1[:, :])
                    for i in range(npair):
                        pr = probs_pool.tile([128, 1024], BF16, tag="pr")
                        if i % 4 == 2:
                            nc.vector.tensor_scalar(
                                out=pr.bitcast(I16), in0=sc_cur,
                                scalar1=128.0, scalar2=B_EXP2,
                                op0=mybir.AluOpType.mult, op1=mybir.AluOpType.add,
                            )
                        else:
                            nc.scalar.activation(
                                out=pr, in_=sc_cur,
                                func=mybir.ActivationFunctionType.Exp,
                                scale=LN2,
                            )
                        if i + 1 < npair:
                            sc_cur = emit_scores(qc, phase, i + 1)
                        elif si + 1 < len(seq):
                            nqc, nph = seq[si + 1]
                            sc_cur = emit_scores(nqc, nph, 0)
                        pump(pump_n)
                        if phase == "h01":
                            drain_until(vproj_done, i // 4)
                            nc.tensor.matmul(
                                acc0, v_c[i // 4][:, i % 4, 0, :], pr[:, 0:512],
                                start=(i == 0), stop=(i == npair - 1),
                                skip_group_check=True,
                            )
                            nc.tensor.matmul(
                                acc1, v_c[i // 4][:, i % 4, 1, :], pr[:, 512:1024],
                                start=(i == 0), stop=(i == npair - 1),
                                skip_group_check=True,
                            )
                        else:
                            t0, t1 = 2 * i, 2 * i + 1
                            drain_until(vproj_done, t0 // 4)
                            drain_until(vproj_done, t1 // 4)
                            nc.tensor.matmul(
                                acc2, v_c[t0 // 4][:, t0 % 4, 2, :], pr[:, 0:512],
                                start=(i == 0), stop=False, skip_group_check=True,
                            )
                            nc.tensor.matmul(
                                acc2, v_c[t1 // 4][:, t1 % 4, 2, :], pr[:, 512:1024],
                                start=False, stop=(i == npair - 1),
                                skip_group_check=True,
                            )
                    if phase == "h01":
                        den_st = epi_sb.tile([65, 512], F32, tag="den", name="den")
                        nc.vector.memset(den_st, 1.0)
                        nc.vector.tensor_copy(den_st[0:1, :], acc0[64:65, :])
                        nc.vector.tensor_copy(den_st[32:33, :], acc1[64:65, :])
                        cat0 = epi_sb.tile([128, 512], BF16, tag="cat0", name="cat0")
                        nc.vector.tensor_copy(cat0[0:64, :], acc0[0:64, :])
                        nc.vector.tensor_copy(cat0[64:128, :], acc1[0:64, :])
                        den_cur[0] = den_st
                        cat0_cur[0] = cat0
                    else:
                        den_st = den_cur[0]
                        nc.vector.tensor_copy(den_st[64:65, :], acc2[64:65, :])
                        cat1 = epi_sb.tile([64, 512], BF16, tag="cat1", name="cat1")
                        nc.vector.tensor_copy(cat1, acc2[0:64, :])
                        fq.append(epi_norm_units(qc, den_st, cat0_cur[0], cat1))
                        fq.append(epi_out_units(qc))

                pump(10**9)

    if split:
        split_multi_waits(nc)
    return nc


_NC_CACHE = {}


def _get_nc(S, has_bq, has_bk):
    key = (S, has_bq, has_bk)
    if key not in _NC_CACHE:
        _NC_CACHE[key] = build_nc(S, has_bq, has_bk)
    return _NC_CACHE[key]


def _arrange_x(X, S):
    """[S, D] input -> [S//512, 128, KCH, 512] chunk-contiguous bf16 layout
    (one contiguous 6KB run per (chunk, partition) for single-descriptor-
    per-partition DMAs).  arr[c, p, cc, s] = X.T[cc*128+p, c*512+s]."""
    xt = X.T.astype(_bf16np)  # [D, S]
    return np.ascontiguousarray(
        xt.reshape(KCH, 128, S // 512, 512).transpose(2, 1, 0, 3)
    )


def _arrange_w(Wslice):
    """[DH, D] weight slice -> [128, KCH, DH] bf16: w[p, cc, n] =
    W.T[cc*128+p, n]."""
    wt = Wslice.T.astype(_bf16np)  # [D, DH]
    return np.ascontiguousarray(wt.reshape(KCH, 128, DH).transpose(1, 0, 2))


def shard_inputs(Q, K, V, Wq, bq, Wk, bk, Wv, bv, Wo, bo, S):
    """Build the 8 per-core input maps (numpy, host-side shard+cast)."""
    has_bq = bool(np.any(bq))
    has_bk = bool(np.any(bk))
    qscale = LOG2E / 8.0  # 1/sqrt(dk) plus the exp2-trick log2e factor
    in_maps = []
    xq_by_batch = [_arrange_x(Q[b], S) for b in range(Q.shape[0])]
    xk_by_batch = [_arrange_x(K[b], S) for b in range(Q.shape[0])]
    xv_by_batch = [_arrange_x(V[b], S) for b in range(Q.shape[0])]
    for c in range(8):
        b = c // 4
        r0 = 3 * (c % 4) * 64
        rows = slice(r0, r0 + DH)
        m = {
            "xqa": xq_by_batch[b],
            "xka": xk_by_batch[b],
            "xva": xv_by_batch[b],
            "wqa": _arrange_w(Wq[rows] * qscale),
            "wka": _arrange_w(Wk[rows]),
            "wva": _arrange_w(Wv[rows]),
            "wo0": np.ascontiguousarray(Wo[:, rows][:, 0:128].T).astype(_bf16np),
            "wo1": np.ascontiguousarray(Wo[:, rows][:, 128:DH].T).astype(_bf16np),
        }
        if has_bq:
            m["bq"] = (bq[rows] * qscale).reshape(DH, 1).astype(np.float32)
        if has_bk:
            m["bk"] = bk[rows].reshape(DH, 1).astype(np.float32)
        in_maps.append(m)
    return in_maps


def gather_output(results, Q, bv, Wo, bo):
    B, S = Q.shape[0], Q.shape[1]
    out = np.zeros((B, S, D_MODEL), np.float32)
    for c, r in enumerate(results):
        out[c // 4] += r["part"].astype(np.float32).T
    out += (bv.astype(np.float32) @ Wo.T.astype(np.float32) + bo.astype(np.float32))[
        None, None, :
    ]
    return out


def kernel(Q, K, V, Wq, bq, Wk, bk, Wv, bv, Wo, bo, **run_kwargs):
    Q, K, V, Wq, bq, Wk, bk, Wv, bv, Wo, bo = (
        np.asarray(a) for a in (Q, K, V, Wq, bq, Wk, bk, Wv, bv, Wo, bo)
    )
    S = Q.shape[1]
    nc = _get_nc(S, bool(np.any(bq)), bool(np.any(bk)))
    in_maps = shard_inputs(Q, K, V, Wq, bq, Wk, bk, Wv, bv, Wo, bo, S)
    res = run_bass_kernel_spmd(nc, in_maps, core_ids=list(range(8)), **run_kwargs)
    out = gather_output(res.results, Q, bv, Wo, bo)
    kernel.last_results = res
    return out


# revision 7
# speedup vs baseline: 1.0141x; 1.0141x over previous
"""Multi-head attention (B=2, S=4096, D=768, H=12) on 8 Trainium2 cores.

Sharding: (batch, head-group) -> core.  Core c handles batch c//4 and heads
3*(c%4) .. 3*(c%4)+2.  Q/K/V projections are computed per-core on the head
slice of the weights; the output projection is computed as a partial product
over the core's 192 combined-head dims and the 4 partials per batch are summed
on the host (the "all-reduce").

Device-side design (v2):
  - PE stream-rows are the floor (~410us of matmul streaming); ACT exp was
    the secondary floor (427us).  v2 attacks both:
      * ~25% of the softmax exp tiles move to the Vector engine via a
        one-instruction Schraudolph exp2 bit-trick: scores carry
        y = log2e * s (log2e/8 folded into Wq), probs_bf16 = bitcast_i16(
        y*128 + 16248.75).  ACT tiles compute exp(y*ln2) via scale=ln2.
        rms rel err of the trick is 1.8%; at 25% of tiles it adds ~0.9%
        in quadrature to the output.
      * the epilogue is transpose-free: attn^T [dk,q] tiles are normalized
        in place by broadcasting 1/den across partitions with tiny K=1
        PE matmuls (den rows staged to SBUF by DMA, one native reciprocal
        per 512-chunk), and the output projection runs transposed
        (out^T[od,s] = WoT-chunk @ attn^T) writing partT [768, S].
        This kills all 160 PE transposes and most epilogue DVE traffic.
      * q/k projections stream N=512 (vs 256) so LDWEIGHTS hides.
  - Everything else is inherited from v1: flat (qc, h, pair) loop with
    cross-boundary scores prefetch, filler pump for projections/epilogue,
    3 parallel DMA queues for inputs, row-packed scores via tile_position,
    v carries a ones-column so attn^T row 64 accumulates the denominator.
  - PSUM: 2x scores(2 banks) + 2x acc(1) + 2x misc(1) = 8 banks.
"""

import os
import sys
from collections import deque

import numpy as np

for _p in ("/opt/trn_rl_repo", "/root/.axon_site/_ro/trn_rl_repo"):
    if _p not in sys.path and os.path.isdir(_p):
        sys.path.append(_p)

import concourse.bass as bass
import concourse.mybir as mybir
import concourse.tile as tile
from concourse.bass_utils import run_bass_kernel_spmd

try:
    from ml_dtypes import bfloat16 as _bf16np
except ImportError:  # pragma: no cover
    _bf16np = np.dtype("bfloat16").type

F32 = mybir.dt.float32
BF16 = mybir.dt.bfloat16
I16 = mybir.dt.int16

D_MODEL = 768
N_HEADS_CORE = 3  # heads per core
DH = 192  # N_HEADS_CORE * 64
KCH = D_MODEL // 128  # contraction chunks for projections

LOG2E = 1.4426950408889634
LN2 = 0.6931471805599453
B_EXP2 = 127.0 * 128.0 - 7.25  # Schraudolph constant for bf16 exp2 trick
PART_BF16 = True  # bf16 partials halve the output-DMA tail


def split_multi_waits(nc, max_waits=1):
    """This container's walrus rejects >1 semaphore wait per instruction
    (setupSyncWait).  Move excess waits onto same-engine NoOps just before
    the offending instruction."""
    n = 0
    for f in nc.m.functions:
        for bb in f.blocks:
            out = []
            for inst in bb.instructions:
                si = inst.sync_info
                if si is not None and si.on_wait and len(si.on_wait) > max_waits:
                    waits = list(si.on_wait)
                    for j, w in enumerate(waits[:-max_waits]):
                        out.append(
                            mybir.InstNoOp(
                                name=f"{inst.name}-wsplit{j}",
                                engine=inst.engine,
                                ins=[],
                                outs=[],
                                sync_info=mybir.SyncInfo(on_wait=[w], on_update=[]),
                            )
                        )
                    si.on_wait = waits[-max_waits:]
                    n += 1
                out.append(inst)
            bb.instructions = out
    return n


def build_nc(S, has_bq=True, has_bk=True, split=True):
    assert S % 512 == 0
    NQ = S // 512  # query chunks / projection chunks
    NT = S // 128  # kpos tiles
    NP2 = NT // 2  # kpos tile pairs for the h2 phase
    nc = bass.Bass()
    PDT = BF16 if PART_BF16 else F32

    # chunk-contiguous layouts: one 6KB descriptor per partition per chunk
    xqa = nc.declare_dram_parameter("xqa", [S // 512, 128, KCH, 512], BF16, isOutput=False)
    xka = nc.declare_dram_parameter("xka", [S // 512, 128, KCH, 512], BF16, isOutput=False)
    xva = nc.declare_dram_parameter("xva", [S // 512, 128, KCH, 512], BF16, isOutput=False)
    wqa = nc.declare_dram_parameter("wqa", [128, KCH, DH], BF16, isOutput=False)
    wka = nc.declare_dram_parameter("wka", [128, KCH, DH], BF16, isOutput=False)
    wva = nc.declare_dram_parameter("wva", [128, KCH, DH], BF16, isOutput=False)
    wo0 = nc.declare_dram_parameter("wo0", [128, D_MODEL], BF16, isOutput=False)
    wo1 = nc.declare_dram_parameter("wo1", [64, D_MODEL], BF16, isOutput=False)
    bq = nc.declare_dram_parameter("bq", [DH, 1], F32, isOutput=False) if has_bq else None
    bk = nc.declare_dram_parameter("bk", [DH, 1], F32, isOutput=False) if has_bk else None
    part = nc.declare_dram_parameter("part", [D_MODEL, S], PDT, isOutput=True)

    with tile.TileContext(nc) as tc:
        with (
            tc.tile_pool(name="consts", bufs=1) as consts,
            tc.tile_pool(name="persist", bufs=1) as persist,
            tc.tile_pool(name="xin", bufs=1) as xin,
            tc.tile_pool(name="probs", bufs=6) as probs_pool,
            tc.tile_pool(name="epi_sb", bufs=2) as epi_sb,
        ):
            # ---- constants ----
            wq_sb = consts.tile([128, KCH, DH], BF16, tag="wq_sb")
            nc.scalar.dma_start(out=wq_sb, in_=wqa[:, :, :])
            wk_sb = consts.tile([128, KCH, DH], BF16, tag="wk_sb")
            nc.scalar.dma_start(out=wk_sb, in_=wka[:, :, :])
            wv_sb = consts.tile([128, KCH, DH], BF16, tag="wv_sb")
            nc.scalar.dma_start(out=wv_sb, in_=wva[:, :, :])
            bq_lo = bq_hi = bk_lo = bk_hi = None
            if has_bq:
                bq_lo = consts.tile([128, 1], F32, tag="bq_lo")
                nc.scalar.dma_start(out=bq_lo, in_=bq[0:128, :])
                bq_hi = consts.tile([64, 1], F32, tag="bq_hi")
                nc.scalar.dma_start(out=bq_hi, in_=bq[128:DH, :])
            if has_bk:
                bk_lo = consts.tile([128, 1], F32, tag="bk_lo")
                nc.scalar.dma_start(out=bk_lo, in_=bk[0:128, :])
                bk_hi = consts.tile([64, 1], F32, tag="bk_hi")
                nc.scalar.dma_start(out=bk_hi, in_=bk[128:DH, :])
            wo0_sb = consts.tile([128, D_MODEL], BF16, tag="wo0")
            wo1_sb = consts.tile([64, D_MODEL], BF16, tag="wo1")
            # ones rows (at 32-aligned partitions) for the 1/den broadcast mms
            sel = consts.tile([65, 64], BF16, tag="sel")
            nc.vector.memset(sel[0:1, :], 1.0)
            nc.vector.memset(sel[32:33, :], 1.0)
            nc.vector.memset(sel[64:65, :], 1.0)

            # ---- persistent activations, per 512-col chunk ----
            qTr = [[persist.tile([128, 512], BF16, tag=f"qTr{h}_{c}", name=f"qTr{h}_{c}")
                    for c in range(NQ)] for h in range(3)]
            kTr = [[persist.tile([128, 512], BF16, tag=f"kTr{h}_{c}", name=f"kTr{h}_{c}")
                    for c in range(NQ)] for h in range(3)]
            v_c = [persist.tile([128, 4, 3, 65], BF16, tag=f"v_{c}", name=f"v_{c}")
                   for c in range(NQ)]
            for c in range(NQ):
                nc.vector.memset(v_c[c][:, :, :, 64:65], 1.0)

            # per-chunk x views: (tile, column offset within tile)
            xq_t = [None] * NQ
            xk_t = [None] * NQ
            xv_t = [None] * NQ

            def dma_xq(c):
                t = xin.tile([128, KCH, 512], BF16, tag="xq", bufs=2, name=f"xq{c}")
                nc.sync.dma_start(out=t, in_=xqa[c])
                xq_t[c] = (t, 0)

            def dma_xk(c, eng=None):
                t = xin.tile([128, KCH, 512], BF16, tag="xk1", bufs=4, name=f"xk{c}")
                (eng or nc.gpsimd).dma_start(out=t, in_=xka[c])
                xk_t[c] = (t, 0)

            def dma_xv(c, eng=None):
                t = xin.tile([128, KCH, 512], BF16, tag="xv1", bufs=4, name=f"xv{c}")
                (eng or nc.sync).dma_start(out=t, in_=xva[c])
                xv_t[c] = (t, 0)

            with (
                tc.tile_pool(name="ps_big", bufs=2, space="PSUM") as ps_big,
                tc.tile_pool(name="ps_acc", bufs=2, space="PSUM") as ps_acc,
                tc.tile_pool(name="ps_misc", bufs=2, space="PSUM") as ps_misc,
            ):
                # ---------- projection helpers ----------
                def qk_proj_units(w_sb, xv, dst, blo, bhi, c):
                    """One full 512-col q/k projection chunk; N=512 streams so
                    LDWEIGHTS hides behind the previous matmul."""
                    x_t, xoff = xv
                    xc = bass.ds(xoff, 512)
                    ps0 = ps_misc.tile([128, 512], F32, tag="misc", name="ps0")
                    for k0 in range(0, KCH, 2):
                        for kk in (k0, k0 + 1):
                            nc.tensor.matmul(ps0, w_sb[:, kk, 0:128], x_t[:, kk, xc],
                                             start=(kk == 0), stop=(kk == KCH - 1),
                                             skip_group_check=True)
                        yield
                    ps1 = ps_misc.tile([128, 512], F32, tag="misc", name="ps1")
                    for k0 in range(0, KCH, 2):
                        for kk in (k0, k0 + 1):
                            nc.tensor.matmul(ps1[0:64, :], w_sb[:, kk, 128:DH], x_t[:, kk, xc],
                                             start=(kk == 0), stop=(kk == KCH - 1),
                                             skip_group_check=True)
                        yield
                    if blo is not None:
                        nc.vector.tensor_scalar_add(dst[0][c][0:64, :], ps0[0:64, :], blo[0:64])
                        nc.vector.tensor_scalar_add(dst[1][c][64:128, :], ps0[64:128, :], blo[64:128])
                        nc.vector.tensor_scalar_add(dst[2][c][0:64, :], ps1[0:64, :], bhi[0:64])
                    else:
                        nc.vector.tensor_copy(dst[0][c][0:64, :], ps0[0:64, :])
                        nc.vector.tensor_copy(dst[1][c][64:128, :], ps0[64:128, :])
                        nc.vector.tensor_copy(dst[2][c][0:64, :], ps1[0:64, :])
                    nc.sync.dma_start(out=dst[2][c][64:128, :], in_=dst[2][c][0:64, :])
                    yield

                kproj_done = [False] * NQ
                vproj_done = [False] * NQ
                qproj_done = [False] * NQ

                def kproj_units(c):
                    yield from qk_proj_units(wk_sb, xk_t[c], kTr, bk_lo, bk_hi, c)
                    kproj_done[c] = True

                def qproj_units(c):
                    yield from qk_proj_units(wq_sb, xq_t[c], qTr, bq_lo, bq_hi, c)
                    qproj_done[c] = True

                def vproj_units(c):
                    x_t, xoff = xv_t[c]
                    for sub in range(4):
                        vps = ps_misc.tile([128, 512], F32, tag="misc", name="vps")
                        for kk in range(KCH):
                            nc.tensor.matmul(
                                vps[:, 0:DH],
                                x_t[:, kk, bass.ds(xoff + sub * 128, 128)],
                                wv_sb[:, kk, :],
                                start=(kk == 0), stop=(kk == KCH - 1),
                                skip_group_check=True,
                            )
                        nc.vector.tensor_copy(
                            v_c[c][:, sub, :, 0:64],
                            vps[:, 0:DH].rearrange("p (h d) -> p h d", h=3),
                        )
                        yield
                    vproj_done[c] = True

                # ---------- epilogue (transpose-free, two-stage) ----------
                catn_t = {}
                epi_state = {}

                def epi_norm_a_units(qc, den_st, cat0):
                    """h0/h1 normalization — pumped during the h2 phase.
                    Chunked reciprocal so the DVE queue never blocks."""
                    rec = epi_sb.tile([65, 512], F32, tag="rec", name="rec")
                    for j in range(8):
                        js = bass.ts(j, 64)
                        nc.vector.reciprocal(rec[0:33, js], den_st[0:33, js])
                        yield
                    recb = epi_sb.tile([65, 512], BF16, tag="recb", name="recb")
                    nc.vector.tensor_copy(recb[0:33, :], rec[0:33, :])
                    epi_state[qc] = (rec, recb)
                    yield
                    yield
                    bc0 = ps_misc.tile([128, 512], F32, tag="misc", name="bc0")
                    nc.tensor.matmul(bc0[0:64, :], sel[0:1, :], recb[0:1, :],
                                     start=True, stop=True, skip_group_check=True)
                    nc.tensor.matmul(bc0[64:128, :], sel[32:33, :], recb[32:33, :],
                                     start=True, stop=True, skip_group_check=True)
                    yield
                    catn0 = epi_sb.tile([128, 512], BF16, tag="catn0", name="catn0")
                    nc.vector.tensor_tensor(out=catn0, in0=cat0, in1=bc0,
                                            op=mybir.AluOpType.mult)
                    catn_t[qc] = catn0
                    yield

                def epi_norm_b_units(qc, den_st, cat1):
                    rec, recb = epi_state.pop(qc)
                    for j in range(4):
                        js = bass.ts(j, 128)
                        nc.vector.reciprocal(rec[64:65, js], den_st[64:65, js])
                        yield
                    nc.vector.tensor_copy(recb[64:65, :], rec[64:65, :])
                    yield
                    yield
                    bc1 = ps_misc.tile([128, 512], F32, tag="misc", name="bc1")
                    nc.tensor.matmul(bc1[0:64, :], sel[64:65, :], recb[64:65, :],
                                     start=True, stop=True, skip_group_check=True)
                    yield
                    catn1 = epi_sb.tile([64, 512], BF16, tag="catn1", name="catn1")
                    nc.vector.tensor_tensor(out=catn1, in0=cat1, in1=bc1[0:64, :],
                                            op=mybir.AluOpType.mult)
                    catn_t[qc] = (catn_t[qc], catn1)
                    yield

                out_queues = [nc.sync, nc.gpsimd, nc.scalar]
                last_queues = [nc.sync, nc.gpsimd, nc.scalar]

                def epi_out_units(qc):
                    catn0, catn1 = catn_t.pop(qc)
                    qs = last_queues if qc == NQ - 1 else out_queues
                    for t in range(6):
                        opT = ps_misc.tile([128, 512], F32, tag="misc", name="opT")
                        nc.tensor.matmul(opT, wo0_sb[:, bass.ts(t, 128)], catn0,
                                         start=True, stop=False, skip_group_check=True)
                        nc.tensor.matmul(opT, wo1_sb[:, bass.ts(t, 128)], catn1,
                                         start=False, stop=True, skip_group_check=True)
                        o_sb = epi_sb.tile([128, 512], PDT, tag="osb", bufs=4, name="osb")
                        nc.vector.tensor_copy(o_sb, opT)
                        q = qs[(qc * 6 + t) % len(qs)]
                        q.dma_start(
                            out=part[t * 128:(t + 1) * 128, qc * 512:(qc + 1) * 512],
                            in_=o_sb,
                        )
                        yield

                # ---------- scores ----------
                def scores_h01(t, qc):
                    sc = ps_big.tile([128, 1024], F32, tag="big", name="sc")
                    tc_ = bass.ts(t % 4, 128)
                    nc.tensor.matmul(
                        sc[:, 0:512], kTr[0][t // 4][0:64, tc_], qTr[0][qc][0:64, :],
                        start=True, stop=True, tile_position=(0, 0),
                    )
                    nc.tensor.matmul(
                        sc[:, 512:1024], kTr[1][t // 4][64:128, tc_], qTr[1][qc][64:128, :],
                        start=True, stop=True, tile_position=(64, 0),
                    )
                    return sc

                def scores_h2(t2, qc):
                    t0, t1 = 2 * t2, 2 * t2 + 1
                    sc = ps_big.tile([128, 1024], F32, tag="big", name="sc")
                    nc.tensor.matmul(
                        sc[:, 0:512],
                        kTr[2][t0 // 4][0:64, bass.ts(t0 % 4, 128)],
                        qTr[2][qc][0:64, :],
                        start=True, stop=True, tile_position=(0, 0),
                    )
                    nc.tensor.matmul(
                        sc[:, 512:1024],
                        kTr[2][t1 // 4][64:128, bass.ts(t1 % 4, 128)],
                        qTr[2][qc][64:128, :],
                        start=True, stop=True, tile_position=(64, 0),
                    )
                    return sc

                # ---------- filler pump ----------
                fq = deque()

                def pump(n):
                    done = 0
                    while done < n and fq:
                        try:
                            next(fq[0])
                            done += 1
                        except StopIteration:
                            fq.popleft()

                def drain_until(flags, c):
                    while not flags[c] and fq:
                        try:
                            next(fq[0])
                        except StopIteration:
                            fq.popleft()
                    assert flags[c], f"filler queue drained but chunk {c} not emitted"

                # ---------- startup ----------
                dma_xq(0)
                queues = [nc.gpsimd, nc.sync, nc.scalar]
                qi = 0
                for c in range(NQ):
                    dma_xk(c, queues[qi % 3]); qi += 1
                    dma_xv(c, queues[qi % 3]); qi += 1
                for g in (qproj_units(0), kproj_units(0)):
                    for _ in g:
                        pass
                fq.append(vproj_units(0))
                fq.append(kproj_units(1))
                fq.append(vproj_units(1))
                for c in range(2, NQ):
                    fq.append(kproj_units(c))
                    fq.append(vproj_units(c))

                # ---------- flat main loop ----------
                seq = []
                for qc in range(NQ):
                    seq.append((qc, "h01"))
                    seq.append((qc, "h2"))

                sc_cur = None
                den_cur = [None]
                cat0_cur = [None]

                def emit_scores(qc, phase, i):
                    if phase == "h01":
                        drain_until(kproj_done, i // 4)
                        drain_until(qproj_done, qc)
                        return scores_h01(i, qc)
                    drain_until(kproj_done, (2 * i) // 4)
                    drain_until(kproj_done, (2 * i + 1) // 4)
                    drain_until(qproj_done, qc)
                    return scores_h2(i, qc)

                sc_cur = emit_scores(0, "h01", 0)
                for si, (qc, phase) in enumerate(seq):
                    npair = NT if phase == "h01" else NP2
                    if phase == "h01":
                        acc0 = ps_acc.tile([65, 512], F32, tag="acc", name="acc0")
                        acc1 = ps_acc.tile([65, 512], F32, tag="acc", name="acc1")
                        pump_n = 6 if qc == 0 else 1
                        if qc >= 1 and qc + 1 < NQ:
                            dma_xq(qc + 1)
                            fq.append(qproj_units(qc + 1))
                    else:
                        acc2 = ps_acc.tile([65, 512], F32, tag="acc", name="acc2")
                        pump_n = 3 if qc == 0 else 1
                        if qc == 0:
                            dma_xq(1)
                            fq.append(qproj_units(1))
                            nc.sync.dma_start(out=wo0_sb, in_=wo0[:, :])
                            nc.sync.dma_start(out=wo1_sb, in_=wo1[:, :])
                    for i in range(npair):
                        pr = probs_pool.tile([128, 1024], BF16, tag="pr")
                        if i % 4 == 2:
                            nc.vector.tensor_scalar(
                                out=pr.bitcast(I16), in0=sc_cur,
                                scalar1=128.0, scalar2=B_EXP2,
                                op0=mybir.AluOpType.mult, op1=mybir.AluOpType.add,
                            )
                        else:
                            nc.scalar.activation(
                                out=pr, in_=sc_cur,
                                func=mybir.ActivationFunctionType.Exp,
                                scale=LN2,
                            )
                        if i + 1 < npair:
                            sc_cur = emit_scores(qc, phase, i + 1)
                        elif si + 1 < len(seq):
                            nqc, nph = seq[si + 1]
                            sc_cur = emit_scores(nqc, nph, 0)
                        pump(pump_n)
                        if phase == "h01":
                            drain_until(vproj_done, i // 4)
                            nc.tensor.matmul(
                                acc0, v_c[i // 4][:, i % 4, 0, :], pr[:, 0:512],
                                start=(i == 0), stop=(i == npair - 1),
                                skip_group_check=True,
                            )
                            nc.tensor.matmul(
                                acc1, v_c[i // 4][:, i % 4, 1, :], pr[:, 512:1024],
                                start=(i == 0), stop=(i == npair - 1),
                                skip_group_check=True,
                            )
                        else:
                            t0, t1 = 2 * i, 2 * i + 1
                            drain_until(vproj_done, t0 // 4)
                            drain_until(vproj_done, t1 // 4)
                            nc.tensor.matmul(
                                acc2, v_c[t0 // 4][:, t0 % 4, 2, :], pr[:, 0:512],
                                start=(i == 0), stop=False, skip_group_check=True,
                            )
                            nc.tensor.matmul(
                                acc2, v_c[t1 // 4][:, t1 % 4, 2, :], pr[:, 512:1024],
                                start=False, stop=(i == npair - 1),
                                skip_group_check=True,
                            )
                    if phase == "h01":
                        den_st = epi_sb.tile([65, 512], F32, tag="den", name="den")
                        nc.vector.memset(den_st, 1.0)
                        nc.vector.tensor_copy(den_st[0:1, :], acc0[64:65, :])
                        nc.vector.tensor_copy(den_st[32:33, :], acc1[64:65, :])
                        cat0 = epi_sb.tile([128, 512], BF16, tag="cat0", name="cat0")
                        nc.vector.tensor_copy(cat0[0:64, :], acc0[0:64, :])
                        nc.vector.tensor_copy(cat0[64:128, :], acc1[0:64, :])
                        den_cur[0] = den_st
                        fq.append(epi_norm_a_units(qc, den_st, cat0))
                    else:
                        den_st = den_cur[0]
                        nc.vector.tensor_copy(den_st[64:65, :], acc2[64:65, :])
                        cat1 = epi_sb.tile([64, 512], BF16, tag="cat1", name="cat1")
                        nc.vector.tensor_copy(cat1, acc2[0:64, :])
                        fq.append(epi_norm_b_units(qc, den_st, cat1))
                        fq.append(epi_out_units(qc))

                pump(10**9)

    if split:
        split_multi_waits(nc)
    return nc


_NC_CACHE = {}


def _get_nc(S, has_bq, has_bk):
    key = (S, has_bq, has_bk)
    if key not in _NC_CACHE:
        _NC_CACHE[key] = build_nc(S, has_bq, has_bk)
    return _NC_CACHE[key]


def _arrange_x(X, S):
    """[S, D] input -> [S//512, 128, KCH, 512] chunk-contiguous bf16 layout
    (one contiguous 6KB run per (chunk, partition) for single-descriptor-
    per-partition DMAs).  arr[c, p, cc, s] = X.T[cc*128+p, c*512+s]."""
    xt = X.T.astype(_bf16np)  # [D, S]
    return np.ascontiguousarray(
        xt.reshape(KCH, 128, S // 512, 512).transpose(2, 1, 0, 3)
    )


def _arrange_w(Wslice):
    """[DH, D] weight slice -> [128, KCH, DH] bf16: w[p, cc, n] =
    W.T[cc*128+p, n]."""
    wt = Wslice.T.astype(_bf16np)  # [D, DH]
    return np.ascontiguousarray(wt.reshape(KCH, 128, DH).transpose(1, 0, 2))


def shard_inputs(Q, K, V, Wq, bq, Wk, bk, Wv, bv, Wo, bo, S):
    """Build the 8 per-core input maps (numpy, host-side shard+cast)."""
    has_bq = bool(np.any(bq))
    has_bk = bool(np.any(bk))
    qscale = LOG2E / 8.0  # 1/sqrt(dk) plus the exp2-trick log2e factor
    in_maps = []
    xq_by_batch = [_arrange_x(Q[b], S) for b in range(Q.shape[0])]
    xk_by_batch = [_arrange_x(K[b], S) for b in range(Q.shape[0])]
    xv_by_batch = [_arrange_x(V[b], S) for b in range(Q.shape[0])]
    for c in range(8):
        b = c // 4
        r0 = 3 * (c % 4) * 64
        rows = slice(r0, r0 + DH)
        m = {
            "xqa": xq_by_batch[b],
            "xka": xk_by_batch[b],
            "xva": xv_by_batch[b],
            "wqa": _arrange_w(Wq[rows] * qscale),
            "wka": _arrange_w(Wk[rows]),
            "wva": _arrange_w(Wv[rows]),
            "wo0": np.ascontiguousarray(Wo[:, rows][:, 0:128].T).astype(_bf16np),
            "wo1": np.ascontiguousarray(Wo[:, rows][:, 128:DH].T).astype(_bf16np),
        }
        if has_bq:
            m["bq"] = (bq[rows] * qscale).reshape(DH, 1).astype(np.float32)
        if has_bk:
            m["bk"] = bk[rows].reshape(DH, 1).astype(np.float32)
        in_maps.append(m)
    return in_maps


def gather_output(results, Q, bv, Wo, bo):
    B, S = Q.shape[0], Q.shape[1]
    out = np.zeros((B, S, D_MODEL), np.float32)
    for c, r in enumerate(results):
        out[c // 4] += r["part"].astype(np.float32).T
    out += (bv.astype(np.float32) @ Wo.T.astype(np.float32) + bo.astype(np.float32))[
        None, None, :
    ]
    return out


def kernel(Q, K, V, Wq, bq, Wk, bk, Wv, bv, Wo, bo, **run_kwargs):
    Q, K, V, Wq, bq, Wk, bk, Wv, bv, Wo, bo = (
        np.asarray(a) for a in (Q, K, V, Wq, bq, Wk, bk, Wv, bv, Wo, bo)
    )
    S = Q.shape[1]
    nc = _get_nc(S, bool(np.any(bq)), bool(np.any(bk)))
    in_maps = shard_inputs(Q, K, V, Wq, bq, Wk, bk, Wv, bv, Wo, bo, S)
    res = run_bass_kernel_spmd(nc, in_maps, core_ids=list(range(8)), **run_kwargs)
    out = gather_output(res.results, Q, bv, Wo, bo)
    kernel.last_results = res
    return out
